# revision 4
# baseline (speedup 1.0000x reference)
"""LoRA Linear kernel for Trainium2 — 8-core hybrid sharding, bf16.

out = x @ W^T + b + 2.0 * ((x @ lora_B^T) @ lora_A^T)

Sharding grid (t, f) = (4, 2): core c -> token shard c>>1 (2048 tokens),
out-feature half c&1 (2048 features). All matmul operands cast to bf16 on
the host (halves upload bytes; rel-err budget 2e-2 >> bf16's ~5e-3).
Output downloaded as bf16, upcast + assembled on host.

Device side (per core):
  - W half arrives [2048o, 4096k] bf16; xbar DMA-transpose -> wt [128k, 32kb, 2048o]
  - x shard arrives [2048t, 4096k] bf16; xbar DMA-transpose per 256-token
    chunk -> xt [128k, 32kb, 256t]. No PE transposes at all.
  - per t-tile (128 tokens), per o-quarter (512): psum[t,o] accumulates
    32 k-block matmuls (bf16, N=512); lora xr accumulated alongside
    (rhs = lbt [128,16]); stage2 matmul K=17 with lhsT=[xr^T; ones],
    rhs = [2*lora_A^T; b] adds the low-rank term and bias in one shot.
  - psum -> bf16 sbuf copy -> DMA out.
"""

import numpy as np

N_CORES = 8
B_DIM, S_DIM, D_IN, D_OUT = 4, 2048, 4096, 4096
T = B_DIM * S_DIM            # 8192 tokens
TSHARDS, FSHARDS = 4, 2
T_SH = T // TSHARDS          # 2048 tokens per core
O_SH = D_OUT // FSHARDS      # 2048 out features per core
R = 16
P = 128
KB = D_IN // P               # 32 k-blocks
CHUNK = 256                  # tokens per xbar-transposed x chunk
NCH = T_SH // CHUNK          # 8 chunks
TPC = CHUNK // P             # 2 t-tiles per chunk
OQ = O_SH // 512             # 4 o-quarters of 512

_CACHE = {}


def _build_nc():
    import concourse.bacc as bacc
    import concourse.mybir as mybir
    import concourse.tile as tile
    from concourse.masks import make_identity

    BF16 = mybir.dt.bfloat16
    F32 = mybir.dt.float32

    nc = bacc.Bacc(target_bir_lowering=False)
    x_d = nc.dram_tensor("x", [D_IN, T_SH], BF16, kind="ExternalInput")
    w_d = nc.dram_tensor("w", [D_IN, O_SH], BF16, kind="ExternalInput")
    la_d = nc.dram_tensor("la", [R + 1, O_SH], BF16, kind="ExternalInput")
    lb_d = nc.dram_tensor("lb", [P, KB * R], BF16, kind="ExternalInput")
    out_d = nc.dram_tensor("out", [T_SH, O_SH], BF16, kind="ExternalOutput")

    out_t = out_d[:].rearrange("(tt p) o -> p tt o", p=P)  # [128, 16, 2048]
    x_t3 = None  # set below once nc exists

    with tile.TileContext(nc) as tc:
        with (
            tc.tile_pool(name="const", bufs=1) as const,
            tc.tile_pool(name="xt", bufs=3) as xtp,
            tc.tile_pool(name="osb", bufs=5) as osb_pool,
            tc.tile_pool(name="xr", bufs=6) as xrp,
            tc.tile_pool(name="ps_o", bufs=4, space="PSUM") as ps_o,
            tc.tile_pool(name="ps_r", bufs=2, space="PSUM") as ps_r,
            tc.tile_pool(name="ps_t", bufs=2, space="PSUM") as ps_t,
        ):
            ident = const.tile([P, P], BF16)
            make_identity(nc, ident)

            # resident weights: W^T via xbar DMA transpose, one tile per
            # o-quarter (xbar dest must be contiguous SBUF). The first
            # x-chunk and first W-quarter are split along kb so the first
            # k-block matmuls can start before the full tiles land; DMA
            # issue order puts them ahead of the other W quarters.
            wt_q = [
                const.tile([P, KB, 512], BF16, name=f"wt{oq}", tag=f"wt{oq}")
                for oq in range(OQ)
            ]
            lbt = const.tile([P, KB, R], BF16)
            lat2 = const.tile([R + 1, O_SH], BF16)
            # x^T and W^T are pre-transposed on the host, so every load is
            # a regular strided DMA (no xbar). Issue order: x chunk 0,
            # W quarter 0 (kb-split so the first k-blocks land early),
            # x chunk 1, then the remaining W quarters.
            x_t3 = x_d[:].rearrange("(kb p) t -> p kb t", p=P)
            w_t3 = w_d[:].rearrange("(kb p) o -> p kb o", p=P)
            xt_pre = [xtp.tile([P, KB, CHUNK], BF16, name=f"xtp{i}", tag="xt")
                      for i in range(2)]
            # tiny lora consts first -- the first k-loop's lora matmul
            # waits on lbt, so it must not queue behind the big loads
            nc.sync.dma_start(lbt.rearrange("p kb r -> p (kb r)"), lb_d[:])
            nc.sync.dma_start(lat2, la_d[:])
            KQ = KB // 4
            for q in range(4):
                ksl = slice(q * KQ, (q + 1) * KQ)
                nc.sync.dma_start(xt_pre[0][:, ksl, :], x_t3[:, ksl, :CHUNK])
                nc.sync.dma_start(wt_q[0][:, ksl, :], w_t3[:, ksl, 0:512])
            nc.sync.dma_start(xt_pre[1], x_t3[:, :, CHUNK:2 * CHUNK])
            for oq in range(1, OQ):
                nc.sync.dma_start(
                    wt_q[oq], w_t3[:, :, oq * 512:(oq + 1) * 512]
                )

            # ---- per-tile work helpers ----
            def kloop(xt, tsl, oq, psr=None):
                pso = ps_o.tile([P, 512], F32, tag="pso", name="pso")
                for j in range(KB):
                    nc.tensor.matmul(
                        pso, xt[:, j, tsl], wt_q[oq][:, j, :],
                        start=(j == 0), stop=False,
                    )
                    if psr is not None:
                        nc.tensor.matmul(
                            psr, xt[:, j, tsl], lbt[:, j, :],
                            start=(j == 0), stop=(j == KB - 1),
                        )
                return pso

            def xr_chain(psr):
                # psum -> sbuf bf16 (ones in col 16) -> PE transpose ->
                # [17,128] with ones row for the bias fold
                xr_sb = xrp.tile([P, R + 1], BF16, tag="xra", name="xra")
                nc.any.tensor_copy(out=xr_sb[:, :R], in_=psr)
                nc.any.memset(xr_sb[:, R:R + 1], 1.0)
                pst = ps_t.tile([R + 1, P], BF16, tag="pst", name="pst")
                nc.tensor.transpose(pst, xr_sb, ident)
                xrT = xrp.tile([R + 1, P], BF16, tag="xrt", name="xrt")
                nc.any.tensor_copy(out=xrT, in_=pst)
                return xrT

            def finish(pso, xrT, osb, oq):
                # stage2 matmul folds the low-rank term + bias, then copy
                nc.tensor.matmul(
                    pso, xrT, lat2[:, oq * 512:(oq + 1) * 512],
                    start=False, stop=True,
                )
                nc.any.tensor_copy(
                    out=osb[:, oq * 512:(oq + 1) * 512], in_=pso,
                )

            # ---- first two chunks: oq-major so the PE only needs W
            # quarter oq while later quarters are still streaming in ----
            pre_tiles = [(c, t) for c in range(2) for t in range(TPC)]
            pre_xrT = {}
            pre_osb = {}
            for c, t in pre_tiles:
                pre_osb[(c, t)] = osb_pool.tile(
                    [P, O_SH], BF16, tag="osb", name="osb")
            for oq in range(OQ):
                for c, t in pre_tiles:
                    tsl = slice(t * P, (t + 1) * P)
                    if oq == 0:
                        psr = ps_r.tile([P, R], F32, tag="psr", name="psr")
                        pso = kloop(xt_pre[c], tsl, 0, psr)
                        pre_xrT[(c, t)] = xr_chain(psr)
                    else:
                        pso = kloop(xt_pre[c], tsl, oq)
                    finish(pso, pre_xrT[(c, t)], pre_osb[(c, t)], oq)
                    if oq == OQ - 1:
                        nc.scalar.dma_start(
                            out_t[:, c * TPC + t, :], pre_osb[(c, t)])

            # ---- remaining chunks: fused k-loop (steady state) ----
            # one stationary load per k-block feeds all 4 o-quarters plus
            # the lora matmul (4x fewer LDWEIGHTS than per-quarter loops)
            for ch in range(2, NCH):
                xt = xtp.tile([P, KB, CHUNK], BF16, tag="xt")
                nc.sync.dma_start(
                    xt, x_t3[:, :, ch * CHUNK:(ch + 1) * CHUNK]
                )
                for ti in range(TPC):
                    tt = ch * TPC + ti
                    tsl = slice(ti * P, (ti + 1) * P)
                    psr = ps_r.tile([P, R], F32, tag="psr", name="psr")
                    psos = [
                        ps_o.tile([P, 512], F32, tag="pso", name="pso")
                        for _ in range(OQ)
                    ]
                    for j in range(KB):
                        for oq in range(OQ):
                            nc.tensor.matmul(
                                psos[oq], xt[:, j, tsl], wt_q[oq][:, j, :],
                                start=(j == 0), stop=False,
                            )
                        nc.tensor.matmul(
                            psr, xt[:, j, tsl], lbt[:, j, :],
                            start=(j == 0), stop=(j == KB - 1),
                        )
                    xrT = xr_chain(psr)
                    osb = osb_pool.tile([P, O_SH], BF16, tag="osb", name="osb")
                    for oq in range(OQ):
                        finish(psos[oq], xrT, osb, oq)
                    nc.scalar.dma_start(out_t[:, tt, :], osb)

    nc.compile()
    return nc


def _get_nc():
    if "nc" not in _CACHE:
        _CACHE["nc"] = _build_nc()
    return _CACHE["nc"]


def _make_in_maps(inputs):
    import ml_dtypes

    bf16 = np.dtype(ml_dtypes.bfloat16)
    x = np.asarray(inputs["x"], dtype=np.float32).reshape(T, D_IN)
    W = np.asarray(inputs["W"], dtype=np.float32)
    b = np.asarray(inputs["b"], dtype=np.float32)
    la = np.asarray(inputs["lora_A"], dtype=np.float32)
    lb = np.asarray(inputs["lora_B"], dtype=np.float32)

    x_bf = np.ascontiguousarray(x).astype(bf16)
    w_bf = np.ascontiguousarray(W).astype(bf16)
    xT_bf = np.ascontiguousarray(x_bf.T)   # [D_IN, T]
    wT_bf = np.ascontiguousarray(w_bf.T)   # [D_IN, D_OUT]
    # lbt[p, kb*16+r] = lora_B[r, kb*128+p]
    lbt = np.ascontiguousarray(
        lb.reshape(R, KB, P).transpose(2, 1, 0).reshape(P, KB * R)
    ).astype(bf16)

    in_maps = []
    for c in range(N_CORES):
        ts, oh = c >> 1, c & 1
        osl = slice(oh * O_SH, (oh + 1) * O_SH)
        lat2 = np.concatenate(
            [2.0 * la[osl].T, b[osl][None, :]], axis=0
        ).astype(bf16)
        in_maps.append({
            "x": np.ascontiguousarray(xT_bf[:, ts * T_SH:(ts + 1) * T_SH]),
            "w": np.ascontiguousarray(wT_bf[:, osl]),
            "la": np.ascontiguousarray(lat2),
            "lb": lbt,
        })
    return in_maps


def kernel(x, W, b, lora_A, lora_B):
    from concourse.bass_utils import run_bass_kernel_spmd

    nc = _get_nc()
    in_maps = _make_in_maps(
        {"x": x, "W": W, "b": b, "lora_A": lora_A, "lora_B": lora_B}
    )
    res = run_bass_kernel_spmd(nc, in_maps, core_ids=list(range(N_CORES)))
    out = np.empty((T, D_OUT), dtype=np.float32)
    for c in range(N_CORES):
        ts, oh = c >> 1, c & 1
        out[ts * T_SH:(ts + 1) * T_SH, oh * O_SH:(oh + 1) * O_SH] = (
            res.results[c]["out"].astype(np.float32)
        )
    return out.reshape(B_DIM, S_DIM, D_OUT)


# revision 5
# speedup vs baseline: 1.0040x; 1.0040x over previous
"""LoRA Linear kernel for Trainium2 — 8-core hybrid sharding, bf16.

out = x @ W^T + b + 2.0 * ((x @ lora_B^T) @ lora_A^T)

Sharding grid (t, f) = (4, 2): core c -> token shard c>>1 (2048 tokens),
out-feature half c&1 (2048 features). All matmul operands cast to bf16 on
the host (halves upload bytes; rel-err budget 2e-2 >> bf16's ~5e-3).
Output downloaded as bf16, upcast + assembled on host.

Device side (per core):
  - W half arrives [2048o, 4096k] bf16; xbar DMA-transpose -> wt [128k, 32kb, 2048o]
  - x shard arrives [2048t, 4096k] bf16; xbar DMA-transpose per 256-token
    chunk -> xt [128k, 32kb, 256t]. No PE transposes at all.
  - per t-tile (128 tokens), per o-quarter (512): psum[t,o] accumulates
    32 k-block matmuls (bf16, N=512); lora xr accumulated alongside
    (rhs = lbt [128,16]); stage2 matmul K=17 with lhsT=[xr^T; ones],
    rhs = [2*lora_A^T; b] adds the low-rank term and bias in one shot.
  - psum -> bf16 sbuf copy -> DMA out.
"""

import numpy as np

N_CORES = 8
B_DIM, S_DIM, D_IN, D_OUT = 4, 2048, 4096, 4096
T = B_DIM * S_DIM            # 8192 tokens
TSHARDS, FSHARDS = 4, 2
T_SH = T // TSHARDS          # 2048 tokens per core
O_SH = D_OUT // FSHARDS      # 2048 out features per core
R = 16
P = 128
KB = D_IN // P               # 32 k-blocks
CHUNK = 256                  # tokens per xbar-transposed x chunk
NCH = T_SH // CHUNK          # 8 chunks
TPC = CHUNK // P             # 2 t-tiles per chunk
OQ = O_SH // 512             # 4 o-quarters of 512

_CACHE = {}


def _build_nc():
    import concourse.bacc as bacc
    import concourse.mybir as mybir
    import concourse.tile as tile
    from concourse.masks import make_identity

    BF16 = mybir.dt.bfloat16
    F32 = mybir.dt.float32

    nc = bacc.Bacc(target_bir_lowering=False)
    x_d = nc.dram_tensor("x", [D_IN, T_SH], BF16, kind="ExternalInput")
    w_d = nc.dram_tensor("w", [D_IN, O_SH], BF16, kind="ExternalInput")
    la_d = nc.dram_tensor("la", [R + 1, O_SH], BF16, kind="ExternalInput")
    lb_d = nc.dram_tensor("lb", [P, KB * R], BF16, kind="ExternalInput")
    out_d = nc.dram_tensor("out", [T_SH, O_SH], BF16, kind="ExternalOutput")

    out_t = out_d[:].rearrange("(tt p) o -> p tt o", p=P)  # [128, 16, 2048]
    x_t3 = None  # set below once nc exists

    with tile.TileContext(nc) as tc:
        with (
            tc.tile_pool(name="const", bufs=1) as const,
            tc.tile_pool(name="xt", bufs=3) as xtp,
            tc.tile_pool(name="osb", bufs=5) as osb_pool,
            tc.tile_pool(name="xr", bufs=6) as xrp,
            tc.tile_pool(name="ps_o", bufs=4, space="PSUM") as ps_o,
            tc.tile_pool(name="ps_r", bufs=2, space="PSUM") as ps_r,
            tc.tile_pool(name="ps_t", bufs=2, space="PSUM") as ps_t,
        ):
            ident = const.tile([P, P], BF16)
            make_identity(nc, ident)

            # resident weights: W^T via xbar DMA transpose, one tile per
            # o-quarter (xbar dest must be contiguous SBUF). The first
            # x-chunk and first W-quarter are split along kb so the first
            # k-block matmuls can start before the full tiles land; DMA
            # issue order puts them ahead of the other W quarters.
            wt_q = [
                const.tile([P, KB, 512], BF16, name=f"wt{oq}", tag=f"wt{oq}")
                for oq in range(OQ)
            ]
            lbt = const.tile([P, KB, R], BF16)
            lat2 = const.tile([R + 1, O_SH], BF16)
            # x^T and W^T are pre-transposed on the host, so every load is
            # a regular strided DMA (no xbar). Issue order: x chunk 0,
            # W quarter 0 (kb-split so the first k-blocks land early),
            # x chunk 1, then the remaining W quarters.
            x_t3 = x_d[:].rearrange("(kb p) t -> p kb t", p=P)
            w_t3 = w_d[:].rearrange("(kb p) o -> p kb o", p=P)
            xt_pre = [xtp.tile([P, KB, CHUNK], BF16, name=f"xtp{i}", tag="xt")
                      for i in range(2)]
            # tiny lora consts first -- the first k-loop's lora matmul
            # waits on lbt, so it must not queue behind the big loads
            nc.sync.dma_start(lbt.rearrange("p kb r -> p (kb r)"), lb_d[:])
            nc.sync.dma_start(lat2, la_d[:])
            KQ = KB // 4
            for q in range(4):
                ksl = slice(q * KQ, (q + 1) * KQ)
                nc.sync.dma_start(xt_pre[0][:, ksl, :], x_t3[:, ksl, :CHUNK])
                nc.sync.dma_start(wt_q[0][:, ksl, :], w_t3[:, ksl, 0:512])
            nc.sync.dma_start(xt_pre[1], x_t3[:, :, CHUNK:2 * CHUNK])
            for oq in range(1, OQ):
                nc.sync.dma_start(
                    wt_q[oq], w_t3[:, :, oq * 512:(oq + 1) * 512]
                )

            # ---- per-tile work helpers ----
            def kloop(xt, tsl, oq, psr=None):
                pso = ps_o.tile([P, 512], F32, tag="pso", name="pso")
                for j in range(KB):
                    nc.tensor.matmul(
                        pso, xt[:, j, tsl], wt_q[oq][:, j, :],
                        start=(j == 0), stop=False,
                    )
                    if psr is not None:
                        nc.tensor.matmul(
                            psr, xt[:, j, tsl], lbt[:, j, :],
                            start=(j == 0), stop=(j == KB - 1),
                        )
                return pso

            def xr_chain(psr):
                # psum -> sbuf bf16 (ones in col 16) -> PE transpose ->
                # [17,128] with ones row for the bias fold
                xr_sb = xrp.tile([P, R + 1], BF16, tag="xra", name="xra")
                nc.any.tensor_copy(out=xr_sb[:, :R], in_=psr)
                nc.any.memset(xr_sb[:, R:R + 1], 1.0)
                pst = ps_t.tile([R + 1, P], BF16, tag="pst", name="pst")
                nc.tensor.transpose(pst, xr_sb, ident)
                xrT = xrp.tile([R + 1, P], BF16, tag="xrt", name="xrt")
                nc.any.tensor_copy(out=xrT, in_=pst)
                return xrT

            def finish(pso, xrT, osb, oq):
                # stage2 matmul folds the low-rank term + bias, then copy
                nc.tensor.matmul(
                    pso, xrT, lat2[:, oq * 512:(oq + 1) * 512],
                    start=False, stop=True,
                )
                nc.any.tensor_copy(
                    out=osb[:, oq * 512:(oq + 1) * 512], in_=pso,
                )

            # ---- first two chunks: oq-major so the PE only needs W
            # quarter oq while later quarters are still streaming in ----
            pre_tiles = [(c, t) for c in range(2) for t in range(TPC)]
            pre_xrT = {}
            pre_osb = {}
            for c, t in pre_tiles:
                pre_osb[(c, t)] = osb_pool.tile(
                    [P, O_SH], BF16, tag="osb", name="osb")
            for oq in range(OQ):
                for c, t in pre_tiles:
                    tsl = slice(t * P, (t + 1) * P)
                    if oq == 0:
                        psr = ps_r.tile([P, R], F32, tag="psr", name="psr")
                        pso = kloop(xt_pre[c], tsl, 0, psr)
                        pre_xrT[(c, t)] = xr_chain(psr)
                    else:
                        pso = kloop(xt_pre[c], tsl, oq)
                    finish(pso, pre_xrT[(c, t)], pre_osb[(c, t)], oq)
                    if oq == OQ - 1:
                        nc.scalar.dma_start(
                            out_t[:, c * TPC + t, :], pre_osb[(c, t)])

            # ---- remaining chunks: fused k-loop (steady state) ----
            # one stationary load per k-block feeds all 4 o-quarters plus
            # the lora matmul (4x fewer LDWEIGHTS than per-quarter loops)
            for ch in range(2, NCH):
                xt = xtp.tile([P, KB, CHUNK], BF16, tag="xt")
                nc.sync.dma_start(
                    xt, x_t3[:, :, ch * CHUNK:(ch + 1) * CHUNK]
                )
                # last chunk: run the (cheap) lora k-loops + xr chains up
                # front so they hide under the main k-loops instead of
                # sitting exposed after the final one
                last = ch == NCH - 1
                early_xrT = {}
                if last:
                    for ti in range(TPC):
                        tsl = slice(ti * P, (ti + 1) * P)
                        psr = ps_r.tile([P, R], F32, tag="psr", name="psr")
                        for j in range(KB):
                            nc.tensor.matmul(
                                psr, xt[:, j, tsl], lbt[:, j, :],
                                start=(j == 0), stop=(j == KB - 1),
                            )
                        early_xrT[ti] = xr_chain(psr)
                for ti in range(TPC):
                    tt = ch * TPC + ti
                    tsl = slice(ti * P, (ti + 1) * P)
                    if not last:
                        psr = ps_r.tile([P, R], F32, tag="psr", name="psr")
                    psos = [
                        ps_o.tile([P, 512], F32, tag="pso", name="pso")
                        for _ in range(OQ)
                    ]
                    for j in range(KB):
                        for oq in range(OQ):
                            nc.tensor.matmul(
                                psos[oq], xt[:, j, tsl], wt_q[oq][:, j, :],
                                start=(j == 0), stop=False,
                            )
                        if not last:
                            nc.tensor.matmul(
                                psr, xt[:, j, tsl], lbt[:, j, :],
                                start=(j == 0), stop=(j == KB - 1),
                            )
                    xrT = early_xrT[ti] if last else xr_chain(psr)
                    osb = osb_pool.tile([P, O_SH], BF16, tag="osb", name="osb")
                    for oq in range(OQ):
                        finish(psos[oq], xrT, osb, oq)
                    nc.scalar.dma_start(out_t[:, tt, :], osb)

    nc.compile()
    return nc


def _get_nc():
    if "nc" not in _CACHE:
        _CACHE["nc"] = _build_nc()
    return _CACHE["nc"]


def _make_in_maps(inputs):
    import ml_dtypes

    bf16 = np.dtype(ml_dtypes.bfloat16)
    x = np.asarray(inputs["x"], dtype=np.float32).reshape(T, D_IN)
    W = np.asarray(inputs["W"], dtype=np.float32)
    b = np.asarray(inputs["b"], dtype=np.float32)
    la = np.asarray(inputs["lora_A"], dtype=np.float32)
    lb = np.asarray(inputs["lora_B"], dtype=np.float32)

    x_bf = np.ascontiguousarray(x).astype(bf16)
    w_bf = np.ascontiguousarray(W).astype(bf16)
    xT_bf = np.ascontiguousarray(x_bf.T)   # [D_IN, T]
    wT_bf = np.ascontiguousarray(w_bf.T)   # [D_IN, D_OUT]
    # lbt[p, kb*16+r] = lora_B[r, kb*128+p]
    lbt = np.ascontiguousarray(
        lb.reshape(R, KB, P).transpose(2, 1, 0).reshape(P, KB * R)
    ).astype(bf16)

    in_maps = []
    for c in range(N_CORES):
        ts, oh = c >> 1, c & 1
        osl = slice(oh * O_SH, (oh + 1) * O_SH)
        lat2 = np.concatenate(
            [2.0 * la[osl].T, b[osl][None, :]], axis=0
        ).astype(bf16)
        in_maps.append({
            "x": np.ascontiguousarray(xT_bf[:, ts * T_SH:(ts + 1) * T_SH]),
            "w": np.ascontiguousarray(wT_bf[:, osl]),
            "la": np.ascontiguousarray(lat2),
            "lb": lbt,
        })
    return in_maps


def kernel(x, W, b, lora_A, lora_B):
    from concourse.bass_utils import run_bass_kernel_spmd

    nc = _get_nc()
    in_maps = _make_in_maps(
        {"x": x, "W": W, "b": b, "lora_A": lora_A, "lora_B": lora_B}
    )
    res = run_bass_kernel_spmd(nc, in_maps, core_ids=list(range(N_CORES)))
    out = np.empty((T, D_OUT), dtype=np.float32)
    for c in range(N_CORES):
        ts, oh = c >> 1, c & 1
        out[ts * T_SH:(ts + 1) * T_SH, oh * O_SH:(oh + 1) * O_SH] = (
            res.results[c]["out"].astype(np.float32)
        )
    return out.reshape(B_DIM, S_DIM, D_OUT)
